# revision 4
# baseline (speedup 1.0000x reference)
"""Sort-prefix O(N*F) distributed Bass kernel for AttnLinearEncoder.

Identity: P[i,j] = exp(relu(s_i+d_j)) = max(es_i*ed_j, 1).  Sorting j by
ed descending makes the "exp branch" of every output row a prefix:
{j : es_i*ed_j > 1} = ranks r < k_i, with k_i host-computable.  With
exclusive prefixes U[k] = sum_{r<k} ed_r*z_r and V[k] = sum_{r<k} z_r:

    num[i] = es_i*U[k_i] + (V[N] - V[k_i])
    den[i] = es_i*E[k_i] + (N - k_i)          (host-exact, E = cumsum ed)
    out[i] = softmax(num[i]/den[i] + z[i] + 2b)   [z residual unbiased]

so the O(N^2 F) attention collapses to O(N F) prefix sums + per-row
lookups.  The lookup is done with matmuls: per-128-block inclusive
cumsums (lower-triangular stationary) produce Usb; block-boundary
totals AllGather (8x Wx256 bf16, ~57KB total, replacing the 3.2MB z
gather);
per-core offset table O = Lperm @ slabs; then "one-hot" gather matmuls
whose values carry es_i*rden_i / -rden_i folded in, accumulating
straight into a transposed PSUM accumulator G[f, i] together with the
x@wT residual, so G = z2 with no intermediate evictions.

SPMD uniformity: the sorted sequence is padded with dummy rows (x=0 ->
z=0; bias is folded analytically via num*rden + b) so every 128-row
block holds <=Q=112 output rows and every core owns exactly W aligned
blocks (W=14 for this data).  Output columns use a fixed Q-per-slot
grid padded with dummy columns (host discards).  All data-dependent
structure lives in input DATA (one-hot values, Lperm, x layouts); the
instruction stream is identical across cores.

A 64-matmul warm-up burst with no DMA dependencies leads the PE stream:
the input DMA otherwise gates the first matmuls, the PE never sustains
the ~3.4us of activity the HAM clock gate needs, and the whole kernel
runs at the cold 1.2 GHz K=4/8 rate (~2x slower, measured).  The burst
warms the clock in the DMA shadow.
"""

import numpy as np
import ml_dtypes
from contextlib import ExitStack

import concourse.bass as bass
import concourse.bacc as bacc
import concourse.mybir as mybir
import concourse.tile as tile
from concourse.bass_utils import run_bass_kernel_spmd

FP32 = mybir.dt.float32
BF16 = mybir.dt.bfloat16
BF = ml_dtypes.bfloat16

N_TOTAL = 12288
D = 512
F = 128
NCORES = 8
P = 128
Q = 112                 # output rows per block (column grid)
PK = 512 // Q           # G-accumulator slots per PSUM bank tile
NKC = D // P            # 4 k-chunks of the input dim

# ---------------------------------------------------------------------------
# host plan
# ---------------------------------------------------------------------------


def make_plan(x, v, g, b, att_weights):
    x = np.asarray(x, np.float32)
    v = np.asarray(v, np.float32)
    g = np.asarray(g, np.float32).reshape(F, 1)
    b = np.asarray(b, np.float32).reshape(F)
    aw = np.asarray(att_weights, np.float32).reshape(2 * F)

    w = g * v / np.linalg.norm(v, axis=1, keepdims=True)        # [F, D]
    asad = np.stack([aw[:F], aw[F:]], axis=1)                   # [F, 2]
    sd = (x.astype(np.float64) @ (w.T @ asad).astype(np.float64)
          + (b @ asad).astype(np.float64))                      # [N, 2]
    es = np.exp(sd[:, 0])
    ed = np.exp(sd[:, 1])
    n = x.shape[0]

    perm_j = np.argsort(-ed, kind="stable")
    ed_srt = ed[perm_j]
    k = np.searchsorted(-ed_srt, -1.0 / es, side="left")        # [N] in 0..N
    E = np.concatenate([[0.0], np.cumsum(ed_srt)])
    den = es * E[k] + (n - k)
    rden = 1.0 / den
    esr = es * rden

    # ---- greedy padded layout ----
    # every block starts with one dummy row; <=Q output rows per block.
    order = np.argsort(k, kind="stable")                        # i's by k
    rows = []          # padded sequence: true sorted-rank or -1 (dummy)
    kp = np.zeros(n, np.int64)                                  # padded k'
    blk_cols = []      # per block: list of original-i
    cur_cols = []
    oi = 0             # next i (in `order`) to place
    r = 0              # true rows consumed

    def open_block():
        rows.append(-1)                                         # leading dummy

    open_block()
    while r < n or oi < n:
        # place all i's with k == r at current padded position
        while oi < n and k[order[oi]] == r:
            if len(cur_cols) == Q:                              # block full
                while len(rows) % P:                            # pad rows
                    rows.append(-1)
                blk_cols.append(cur_cols)
                cur_cols = []
                open_block()
            kp[order[oi]] = len(rows)
            cur_cols.append(order[oi])
            oi += 1
        if r < n:
            if len(rows) % P == 0:                              # new block
                blk_cols.append(cur_cols)
                cur_cols = []
                open_block()
            rows.append(perm_j[r])
            r += 1
    while len(rows) % P:
        rows.append(-1)
    blk_cols.append(cur_cols)

    nblk = len(rows) // P
    W = -(-nblk // NCORES)                                      # blocks/core
    nblk_pad = W * NCORES
    while len(blk_cols) < nblk_pad:
        blk_cols.append([])
    rows.extend([-1] * ((nblk_pad - nblk) * P))
    rows = np.asarray(rows, np.int64)
    assert all(len(c) <= Q for c in blk_cols)
    assert rows.size == nblk_pad * P

    RW = W * P            # sorted rows per core
    CAP = W * Q           # output columns per core

    # block index per i (clamped so m in 1..128 uses the earlier block)
    Bi = np.maximum(0, (kp - 1)) // P
    mi = kp - Bi * P                                            # 0..128
    # sanity: each i's Bi equals the block whose column list contains it
    # (by construction)

    xb = np.ascontiguousarray(x.astype(BF))
    wTb = np.ascontiguousarray(w.T.astype(BF))                  # [D, F]
    Ltri = np.tril(np.ones((P, P), np.float32)).T.astype(BF)    # L[k,m]=1 k<=m
    bias2 = np.ascontiguousarray((2.0 * b).reshape(F, 1).astype(np.float32))

    plan = dict(W=W, RW=RW, CAP=CAP, nblk_pad=nblk_pad,
                wT=wTb, Ltri=np.ascontiguousarray(Ltri), bias2=bias2)
    maps, colmaps = [], []
    for c in range(NCORES):
        blo = c * W
        rws = rows[blo * P:(blo + W) * P]                       # [RW]
        xTs = np.zeros((D, RW), BF)
        msk = rws >= 0
        xTs[:, msk] = xb[rws[msk]].T
        eds = np.zeros(RW, np.float32)
        eds[msk] = ed[rws[msk]].astype(np.float32)

        xTo = np.zeros((D, CAP), BF)
        S1u = np.zeros((P, CAP), BF)
        S1v = np.zeros((P, CAP), BF)
        S2u = np.zeros((18, CAP), BF)
        S2v = np.zeros((18, CAP), BF)
        Lp = np.zeros((nblk_pad, 18), BF)
        colmap = np.full(CAP, -1, np.int64)
        for t in range(W):
            gb = blo + t
            Lp[:gb, t] = 1.0                                    # offsets
            cols = blk_cols[gb]
            for j, i in enumerate(cols):
                col = t * Q + j
                colmap[col] = i
                xTo[:, col] = xb[i]
                assert Bi[i] == gb, (c, t, i, Bi[i], gb)
                if mi[i] >= 1:
                    S1u[mi[i] - 1, col] = esr[i]
                    S1v[mi[i] - 1, col] = -rden[i]
                S2u[t, col] = esr[i]
                S2v[t, col] = -rden[i]
                S2v[17, col] = rden[i]
        Lp[:nblk, 17] = 1.0                                     # Vt row
        maps.append({
            "xTs": np.ascontiguousarray(xTs),
            "xTo": np.ascontiguousarray(xTo),
            "eds": np.ascontiguousarray(eds.reshape(W, P).T),   # [P, W]
            "S1u": S1u, "S1v": S1v, "S2u": S2u, "S2v": S2v,
            "Lp1": np.ascontiguousarray(Lp[:min(P, nblk_pad)]),
            "Lp2": np.ascontiguousarray(Lp[min(P, nblk_pad):]),
            "wT": plan["wT"], "Ltri": plan["Ltri"], "bias2": bias2,
        })
        colmaps.append(colmap)
    plan["maps"] = maps
    plan["colmaps"] = colmaps
    return plan


# ---------------------------------------------------------------------------
# device program
# ---------------------------------------------------------------------------


def build(W, nblk_pad, timing_reps=0, rep_which="ab", tlsim=False,
          dma_in_loop=True, warm=64):
    RW, CAP = W * P, W * Q
    LP2 = max(0, nblk_pad - P)                                  # rows in Lp2
    PA = min(P, nblk_pad)                                       # rows in Lp1
    # G tiles: PK slots per PSUM bank-tile, last tile partial
    GT = -(-W // PK)                                            # num G tiles
    gwid = [min(PK, W - PK * g) * Q for g in range(GT)]

    nc = bacc.Bacc("TRN2", target_bir_lowering=False, debug=False,
                   num_devices=1 if tlsim else NCORES)

    xTs_e = nc.dram_tensor("xTs", [D, RW], BF16, kind="ExternalInput")
    xTo_e = nc.dram_tensor("xTo", [D, CAP], BF16, kind="ExternalInput")
    eds_e = nc.dram_tensor("eds", [P, W], FP32, kind="ExternalInput")
    S1u_e = nc.dram_tensor("S1u", [P, CAP], BF16, kind="ExternalInput")
    S1v_e = nc.dram_tensor("S1v", [P, CAP], BF16, kind="ExternalInput")
    S2u_e = nc.dram_tensor("S2u", [18, CAP], BF16, kind="ExternalInput")
    S2v_e = nc.dram_tensor("S2v", [18, CAP], BF16, kind="ExternalInput")
    Lp1_e = nc.dram_tensor("Lp1", [PA, 18], BF16, kind="ExternalInput")
    Lp2_e = (nc.dram_tensor("Lp2", [LP2, 18], BF16, kind="ExternalInput")
             if LP2 else None)
    wT_e = nc.dram_tensor("wT", [D, F], BF16, kind="ExternalInput")
    Ltri_e = nc.dram_tensor("Ltri", [P, P], BF16, kind="ExternalInput")
    bias2_e = nc.dram_tensor("bias2", [F, 1], FP32, kind="ExternalInput")
    out_e = nc.dram_tensor("out", [F, CAP], BF16, kind="ExternalOutput")

    with tile.TileContext(nc) as tc, ExitStack() as ctx:
        const = ctx.enter_context(tc.tile_pool(name="const", bufs=1))
        dram = ctx.enter_context(tc.tile_pool(name="dram", bufs=1, space="DRAM"))
        work = ctx.enter_context(tc.tile_pool(name="work", bufs=1))
        psA_ctx = ExitStack()
        psA = psA_ctx.enter_context(tc.tile_pool(name="psA", bufs=2, space="PSUM"))

        def rep_loop(which):
            if timing_reps <= 0 or which not in rep_which:
                return None
            cm = tc.For_i(0, timing_reps, 1,
                          hint_engines=(mybir.EngineType.PE,
                                        mybir.EngineType.DVE,
                                        mybir.EngineType.Activation,
                                        mybir.EngineType.SP))
            cm.__enter__()
            return cm

        # ---- constants & inputs ----
        wT_sb = const.tile([P, NKC, F], BF16)
        Ltri_sb = const.tile([P, P], BF16)
        bias2_sb = const.tile([F, 1], FP32)
        ones_f = const.tile([P, 1], BF16)
        ones_1 = const.tile([1, P], BF16)
        nc.vector.memset(ones_f[:], 1.0)
        nc.vector.memset(ones_1[:], 1.0)
        nc.gpsimd.dma_start(wT_sb[:], wT_e.ap().rearrange("(c p) f -> p c f", p=P))
        nc.gpsimd.dma_start(Ltri_sb[:], Ltri_e[:])
        nc.gpsimd.dma_start(bias2_sb[:], bias2_e[:])

        rep_a = rep_loop("a") if dma_in_loop else None
        xs_sb = work.tile([P, NKC, RW], BF16, name="xs")
        xo_sb = work.tile([P, NKC, CAP], BF16, name="xo")
        eds_sb = work.tile([P, W], FP32, name="eds")
        S1u_sb = work.tile([P, CAP], BF16, name="S1u")
        S1v_sb = work.tile([P, CAP], BF16, name="S1v")
        S2u_sb = work.tile([18, CAP], BF16, name="S2u")
        S2v_sb = work.tile([18, CAP], BF16, name="S2v")
        Lp1_sb = work.tile([PA, 18], BF16, name="Lp1")
        Lp2_sb = work.tile([LP2, 18], BF16, name="Lp2") if LP2 else None

        xTs_v = xTs_e.ap().rearrange("(c p) i -> p c i", p=P)
        xTo_v = xTo_e.ap().rearrange("(c p) i -> p c i", p=P)
        # split input stream across queues; xTs first (feeds pass A)
        nc.sync.dma_start(xs_sb[:, :, 0:RW // 2],
                          xTs_v[:, :, 0:RW // 2])
        nc.scalar.dma_start(xs_sb[:, :, RW // 2:RW],
                            xTs_v[:, :, RW // 2:RW])
        nc.sync.dma_start(eds_sb[:], eds_e[:])
        nc.scalar.dma_start(xo_sb[:], xTo_v)
        nc.sync.dma_start(S1u_sb[:], S1u_e[:])
        nc.sync.dma_start(S1v_sb[:], S1v_e[:])
        nc.scalar.dma_start(S2u_sb[:], S2u_e[:])
        nc.scalar.dma_start(S2v_sb[:], S2v_e[:])
        nc.sync.dma_start(Lp1_sb[:], Lp1_e[:])
        if LP2:
            nc.sync.dma_start(Lp2_sb[:], Lp2_e[:])

        # collective buffers
        slab_loc = dram.tile([W * 2 * F], BF16, name="slab_loc")
        slab_full = dram.tile([nblk_pad * 2 * F], BF16, addr_space="Shared",
                              name="slab_full")

        movUV = work.tile([P, W, 2 * F], BF16, name="movUV")
        Usb = work.tile([P, W, 2 * F], BF16, name="Usb")

        if rep_a is None:
            rep_a = rep_loop("a")
        if warm:
            wps = psA.tile([P, P], FP32, tag="zps", name="wps")
            for wi in range(warm):
                nc.tensor.matmul(wps[:], Ltri_sb[:], Ltri_sb[:],
                                 start=(wi == 0), stop=(wi == warm - 1))
        # ---- pass A: sorted z, [edz|z], per-block inclusive cumsum ----
        for t4 in range(0, W, 4):
            nt = min(4, W - t4)
            zps = psA.tile([P, nt * F], FP32, tag="zps", name="zps")
            for tl in range(nt):
                t = t4 + tl
                for c in range(NKC):
                    nc.tensor.matmul(zps[:, tl * F:(tl + 1) * F],
                                     xs_sb[:, c, t * P:(t + 1) * P],
                                     wT_sb[:, c, :],
                                     start=(c == 0), stop=(c == NKC - 1))
            # edz on DVE (per-block scalar), z copy on Act (wide)
            for tl in range(nt):
                t = t4 + tl
                nc.vector.tensor_scalar_mul(movUV[:, t, 0:F],
                                            zps[:, tl * F:(tl + 1) * F],
                                            eds_sb[:, t:t + 1])
            nc.scalar.copy(movUV[:, t4:t4 + nt, F:2 * F],
                           zps[:, 0:nt * F].rearrange("p (t f) -> p t f", f=F))
        for t2 in range(0, W, 2):
            nt = min(2, W - t2)
            ups = psA.tile([P, nt * 2 * F], FP32, tag="ups", name="ups")
            for tl in range(nt):
                t = t2 + tl
                nc.tensor.matmul(ups[:, tl * 2 * F:(tl + 1) * 2 * F],
                                 Ltri_sb[:], movUV[:, t, :],
                                 start=True, stop=True)
            eng = nc.vector if (t2 // 2) % 2 == 0 else nc.scalar
            usrc = ups[:, 0:nt * 2 * F].rearrange("p (t f) -> p t f",
                                                  f=2 * F)
            if eng is nc.vector:
                nc.vector.tensor_copy(Usb[:, t2:t2 + nt, :], usrc)
            else:
                nc.scalar.copy(Usb[:, t2:t2 + nt, :], usrc)
        if rep_a is not None:
            rep_a.__exit__(None, None, None)

        # slab: partition-127 rows (block totals) -> DRAM -> AllGather
        nc.sync.dma_start(slab_loc[:],
                          Usb[P - 1:P, :, :].rearrange("p t f -> p (t f)"))
        if tlsim:
            nc.gpsimd.dma_start(slab_full[0:W * 2 * F], slab_loc[:])
        else:
            nc.gpsimd.collective_compute(
                "AllGather", mybir.AluOpType.bypass,
                ins=[slab_loc[:].opt()], outs=[slab_full[:].opt()],
                replica_groups=[list(range(NCORES))])

        rep_b = rep_loop("b")
        # ---- pass B ----
        slabA = work.tile([PA, 2 * F], BF16, name="slabA")
        nc.sync.dma_start(slabA[:], slab_full[0:PA * 2 * F]
                          .rearrange("(r w) -> r w", w=2 * F))
        slabB = work.tile([LP2, 2 * F], BF16, name="slabB") if LP2 else None
        if LP2:
            nc.scalar.dma_start(slabB[:], slab_full[P * 2 * F:nblk_pad * 2 * F]
                                .rearrange("(r w) -> r w", w=2 * F))

        psA_ctx.close()
        psB = ctx.enter_context(tc.tile_pool(name="psB", bufs=1, space="PSUM"))
        G = [psB.tile([P, gwid[gt]], FP32, tag=f"G{gt}", name=f"G{gt}")
             for gt in range(GT)]
        ops = psB.tile([18, 2 * F], FP32, tag="O", name="O")

        # start G with the residual z^T (stationary = wT chunk)
        for gt in range(GT):
            c0, c1 = PK * Q * gt, PK * Q * gt + gwid[gt]
            for c in range(NKC):
                nc.tensor.matmul(G[gt][:], wT_sb[:, c, :],
                                 xo_sb[:, c, c0:c1],
                                 start=(c == 0), stop=False)

        # gather matmuls: S1 per slot (stationary = Usb block halves)
        for t in range(W):
            gt, sl = t // PK, t % PK
            cc = sl * Q
            nc.tensor.matmul(G[gt][:, cc:cc + Q], Usb[:, t, 0:F],
                             S1u_sb[:, t * Q:(t + 1) * Q],
                             start=False, stop=False)
            nc.tensor.matmul(G[gt][:, cc:cc + Q], Usb[:, t, F:2 * F],
                             S1v_sb[:, t * Q:(t + 1) * Q],
                             start=False, stop=False)
        # offset table O = Lperm @ slabs ; evict bf16 ; then S2
        nc.tensor.matmul(ops[:], Lp1_sb[:], slabA[:], start=True, stop=not LP2)
        if LP2:
            nc.tensor.matmul(ops[:], Lp2_sb[:], slabB[:], start=False, stop=True)
        O_sb = work.tile([18, 2 * F], BF16, name="O_sb")
        nc.vector.tensor_copy(O_sb[:], ops[:])
        for gt in range(GT):
            c0, c1 = PK * Q * gt, PK * Q * gt + gwid[gt]
            nc.tensor.matmul(G[gt][:], O_sb[:, 0:F], S2u_sb[:, c0:c1],
                             start=False, stop=False)
            nc.tensor.matmul(G[gt][:], O_sb[:, F:2 * F], S2v_sb[:, c0:c1],
                             start=False, stop=True)

        # ---- epilogue: G == z2^T ; softmax over partitions (f) ----
        e_sb = work.tile([P, CAP], BF16, name="e_sb")
        r_sb = work.tile([1, CAP], BF16, name="r_sb")
        o_sb = work.tile([P, CAP], BF16, name="o_sb")
        for gt in range(GT):
            c0 = PK * Q * gt
            gw = gwid[gt]
            nc.scalar.activation(e_sb[:, c0:c0 + gw], G[gt][:],
                                 mybir.ActivationFunctionType.Exp,
                                 bias=bias2_sb[:])
            sums = psB.tile([1, gw], FP32, tag="O", name=f"sum{gt}")
            nc.tensor.matmul(sums[:], ones_f[:], e_sb[:, c0:c0 + gw],
                             start=True, stop=True)
            with nc.allow_low_precision(reason="row-normalizer in bf16"):
                nc.vector.reciprocal(r_sb[:, c0:c0 + gw], sums[:])
            rbc = psB.tile([P, gw], FP32, tag=f"rb{gt % 2}", name=f"rb{gt}")
            nc.tensor.matmul(rbc[:], ones_1[:], r_sb[:, c0:c0 + gw],
                             start=True, stop=True)
            nc.vector.tensor_mul(o_sb[:, c0:c0 + gw], e_sb[:, c0:c0 + gw],
                                  rbc[:])
            eng = nc.scalar if gt % 2 == 0 else nc.sync
            eng.dma_start(out_e[:, c0:c0 + gw], o_sb[:, c0:c0 + gw])
        if rep_b is not None:
            rep_b.__exit__(None, None, None)

    nc.compile()
    return nc


# ---------------------------------------------------------------------------
# entry
# ---------------------------------------------------------------------------

_CACHE = {}


def kernel(x, v, g, b, att_weights):
    plan = make_plan(x, v, g, b, att_weights)
    key = (plan["W"], plan["nblk_pad"])
    if key not in _CACHE:
        _CACHE[key] = build(plan["W"], plan["nblk_pad"])
    nc = _CACHE[key]
    res = run_bass_kernel_spmd(nc, plan["maps"], core_ids=list(range(NCORES)))
    n = x.shape[0]
    out = np.empty((n, F), np.float32)
    for c in range(NCORES):
        oc = res.results[c]["out"].astype(np.float32)  # [F, CAP]
        cm = plan["colmaps"][c]
        sel = cm >= 0
        out[cm[sel]] = oc[:, sel].T
    return out
